# revision 35
# baseline (speedup 1.0000x reference)
"""Sparse masked attention on 8 TRN2 NeuronCores.

reference:  O = softmax((Q K^T * mq[:,None] + log(mk[None,:])) / 8) @ V
  - rows with mq=0: scores all equal -> uniform average of V over mk=1 keys
  - keys with mk=0: exactly dropped from the softmax

Strategy: batch (b=8) is data-parallel across the 8 cores. On the host we
compact each batch to its unmasked queries/keys (~n/2 each), so each core
computes a dense ~2176x2176 attention instead of 4096x4096:

  S^T[mchunk, n] = (Kc^T chunk).T @ Qc^T      (TensorE bf16, d=64, row-group
                                               pairs: two chunks concurrently)
  P^T = exp(S^T / 8) -> bf16                  (ScalarE exact / VectorE fast-exp)
  O^T[65, n]    += Vext[mchunk].T @ P^T       (TensorE bf16, PSUM accumulate)

where Vext = [V | 1]: the ones column accumulates the softmax denominator.
No row-max subtraction is needed: scores/8 ~ N(0,1), exp stays in range.
The host divides by the denominator, scatters rows back, and fills masked
query rows with mean(V[mk=1]).
"""

import numpy as np
import ml_dtypes

N_CORES = 8
W = 512  # n-block width (PSUM bank / fp32-accum matmul free-dim limit)


def _round_up(x, mult):
    return ((x + mult - 1) // mult) * mult


_build_cache = {}


def _build(ncap, mcap):
    """Per-core graph. Inputs (per core):
      qt   [64, ncap]            bf16   Q^T (compacted, padded)
      ktp  [128, npairs*128]     bf16   K^T chunk pairs: pair p = chunk 2p on
                                        partitions 0-63, chunk 2p+1 on 64-127
      vext [128, mchunks*65]     bf16   partition-major Vext chunks: partition
                                        r, cols [c*65:(c+1)*65] = Vext row
                                        c*128+r = [V row | 1.0] (0 if padding)
    Output: out [nblocks*65, W] f32: block j rows [65j:65j+65], cols [:w_j] =
      [ O^T numerator (64 rows) ; denominator (1 row) ] for n-cols j*W..+w_j.
    """
    key = (ncap, mcap)
    if key in _build_cache:
        return _build_cache[key]

    import concourse.bacc as bacc
    import concourse.mybir as mybir
    import concourse.tile as tile

    f32 = mybir.dt.float32
    bf16 = mybir.dt.bfloat16
    i16 = mybir.dt.int16
    mchunks = mcap // 128
    npairs = (mchunks + 1) // 2
    # balanced n-blocks, each <= W (PSUM bank limit)
    nblocks = (ncap + W - 1) // W
    base, extra = divmod(ncap, nblocks)
    widths = [base + (1 if i < extra else 0) for i in range(nblocks)]
    blocks, off = [], 0
    for wd in widths:
        blocks.append((off, wd))
        off += wd
    EXP = mybir.ActivationFunctionType.Exp

    # Schraudolph fast exp on bf16 bit pattern, via int16:
    #   i16 = (int16)(s * (2^7/ln2)/8 + (127*2^7 - C)) ; bitcast -> bf16
    FEXP_A = float(2 ** 7 / np.log(2.0) / 8.0)
    FEXP_B = float(127 * 2 ** 7 - 7.5)

    nc = bacc.Bacc("TRN2", target_bir_lowering=False, debug=False,
                   num_devices=N_CORES)
    qt_d = nc.dram_tensor("qt", [64, ncap], bf16, kind="ExternalInput")
    ktp_d = nc.dram_tensor("ktp", [128, npairs * 128], bf16,
                           kind="ExternalInput")
    vext_d = nc.dram_tensor("vext", [128, mchunks * 65], bf16,
                            kind="ExternalInput")
    out_d = nc.dram_tensor("out", [len(blocks) * 65, W], f32,
                           kind="ExternalOutput")

    with tile.TileContext(nc) as tc:
        with (
            tc.tile_pool(name="resident", bufs=1) as resident,
            tc.tile_pool(name="pt", bufs=8) as ptp,
            tc.tile_pool(name="osb", bufs=2) as osbp,
            tc.tile_pool(name="psum_s", bufs=4, space="PSUM") as psum_s,
            tc.tile_pool(name="psum_olo", bufs=2, space="PSUM") as psum_olo,
            tc.tile_pool(name="psum_ohi", bufs=2, space="PSUM") as psum_ohi,
        ):
            # sliced DMAs: a small head slice first so compute starts early
            kt_sb = resident.tile([128, npairs * 128], bf16)
            kcut = min(256, npairs * 128)
            nc.sync.dma_start(kt_sb[:, 0:kcut], ktp_d[:, 0:kcut])
            w0 = blocks[0][1]
            qt_sb = resident.tile([128, ncap], bf16)
            nc.sync.dma_start(qt_sb[0:64, 0:w0], qt_d[:, 0:w0])
            nc.sync.dma_start(qt_sb[64:128, 0:w0], qt_d[:, 0:w0])
            v_sb = resident.tile([128, mchunks * 65], bf16)
            vcut = min(130, mchunks * 65)
            nc.sync.dma_start(v_sb[:, 0:vcut], vext_d[:, 0:vcut])
            if kcut < npairs * 128:
                nc.sync.dma_start(kt_sb[:, kcut:], ktp_d[:, kcut:])
            if vcut < mchunks * 65:
                nc.sync.dma_start(v_sb[:, vcut:], vext_d[:, vcut:])
            if ncap > w0:
                nc.sync.dma_start(qt_sb[0:64, w0:], qt_d[:, w0:])
                nc.sync.dma_start(qt_sb[64:128, w0:], qt_d[:, w0:])

            pending_drains = []  # drains of the previous block group

            def drain(jb, olo, ohi, w):
                ohi_sb = osbp.tile([65, w], f32, tag="ohi_sb")
                nc.scalar.copy(ohi_sb[:, :], ohi[:, :])
                osb = osbp.tile([65, w], f32, tag="osb")
                nc.vector.tensor_add(osb[:, :], olo[:, :], ohi_sb[:, :])
                nc.sync.dma_start(out_d[jb * 65:(jb + 1) * 65, 0:w],
                                  osb[:, :])

            def bfc(pt, lo, hi, w):
                ap = pt[lo:hi, :w]
                return ap.bitcast(bf16) if pt.dtype == i16 else ap

            def mm2(s, p, pt_a, pt_b, last):
                # k=64 halves on alternating row groups: chunk c rows 0-63
                # accumulate into olo, rows 64-127 into ohi
                w = s["w"]
                for mi, pt in ((2 * p, pt_a), (2 * p + 1, pt_b)):
                    if pt is None:
                        continue
                    lastc = last and (mi == mchunks - 1)
                    nc.tensor.matmul(
                        s["olo"][:, :], v_sb[0:64, mi * 65:(mi + 1) * 65],
                        bfc(pt, 0, 64, w), start=(mi == 0), stop=lastc,
                        tile_position=(0, 0), skip_group_check=True)
                    nc.tensor.matmul(
                        s["ohi"][:, :], v_sb[64:128, mi * 65:(mi + 1) * 65],
                        bfc(pt, 64, 128, w), start=(mi == 0), stop=lastc,
                        tile_position=(64, 0), skip_group_check=True)

            def exp_act(st, w):
                t = ptp.tile([128, w], bf16, tag="pt")
                nc.scalar.activation(t[:, :], st[:, :], EXP, scale=0.125)
                return t

            def exp_dve(st, w):
                t = ptp.tile([128, w], i16, tag="pt")
                nc.vector.tensor_scalar(
                    t[:, :], st[:, :], FEXP_A, FEXP_B,
                    mybir.AluOpType.mult, mybir.AluOpType.add)
                return t

            # process blocks in pairs: two independent n-streams per pair-step
            # keep the PE fed while each stream waits on its exp
            groups = [blocks[i:i + 2] for i in range(0, len(blocks), 2)]
            for grp in groups:
                streams = []
                for (j0, w) in grp:
                    jb = blocks.index((j0, w))
                    olo = psum_olo.tile([65, w], f32, tag="olo")
                    ohi = psum_ohi.tile([65, w], f32, tag="ohi")
                    streams.append({"jb": jb, "j0": j0, "w": w,
                                    "olo": olo, "ohi": ohi, "prev": None})
                for p in range(npairs):
                    mi1 = 2 * p + 1
                    has_b = mi1 < mchunks
                    sts = []
                    for s in streams:
                        j0, w = s["j0"], s["w"]
                        st_a = psum_s.tile([128, w], f32, tag="st")
                        nc.tensor.matmul(
                            st_a[:, :], kt_sb[0:64, p * 128:(p + 1) * 128],
                            qt_sb[0:64, j0:j0 + w],
                            start=True, stop=True, tile_position=(0, 0))
                        st_b = None
                        if has_b:
                            st_b = psum_s.tile([128, w], f32, tag="st")
                            nc.tensor.matmul(
                                st_b[:, :],
                                kt_sb[64:128, p * 128:(p + 1) * 128],
                                qt_sb[64:128, j0:j0 + w],
                                start=True, stop=True, tile_position=(64, 0))
                        sts.append((st_a, st_b))
                    # previous pair's PV matmuls: queued ahead of the
                    # exp-dependent ones so the PE never idles
                    for s in streams:
                        if s["prev"] is not None:
                            mm2(s, *s["prev"], last=False)
                    # exps: per step each engine gets one tile per stream
                    for i, s in enumerate(streams):
                        st_a, st_b = sts[i]
                        w = s["w"]
                        if has_b:
                            if i % 2 == 0:
                                pt_a, pt_b = exp_act(st_a, w), exp_dve(st_b, w)
                            else:
                                pt_a, pt_b = exp_dve(st_a, w), exp_act(st_b, w)
                        else:
                            pt_a = (exp_act if (i + p) % 2 == 0
                                    else exp_dve)(st_a, w)
                            pt_b = None
                        s["prev"] = (p, pt_a, pt_b)
                    # previous group's output drains, deferred here so they
                    # do not convoy this group's exps in the ACT/DVE FIFOs
                    if p == 0 and pending_drains:
                        for d in pending_drains:
                            drain(*d)
                        pending_drains = []
                for s in streams:
                    mm2(s, *s["prev"], last=True)
                    pending_drains.append((s["jb"], s["olo"], s["ohi"],
                                           s["w"]))
            for d in pending_drains:
                drain(*d)

    nc.compile()
    _build_cache[key] = nc
    return nc


def _run(inputs, trace=False):
    queries = np.asarray(inputs["queries"], dtype=np.float32)
    keys = np.asarray(inputs["keys"], dtype=np.float32)
    values = np.asarray(inputs["values"], dtype=np.float32)
    mask_query = np.asarray(inputs["mask_query"])
    mask_key = np.asarray(inputs["mask_key"])

    b, n, d = queries.shape
    dv = values.shape[2]
    assert b == N_CORES, f"batch {b} != {N_CORES} cores"
    bf = ml_dtypes.bfloat16

    idx_q = [np.flatnonzero(mask_query[i]) for i in range(b)]
    idx_k = [np.flatnonzero(mask_key[i]) for i in range(b)]
    ncap = max(max(len(ix) for ix in idx_q), 64)
    mcap = _round_up(max(max(len(ix) for ix in idx_k), 1), 128)
    mchunks = mcap // 128
    npairs = (mchunks + 1) // 2
    nblocks = (ncap + W - 1) // W
    base, extra = divmod(ncap, nblocks)
    bwidths = [base + (1 if i < extra else 0) for i in range(nblocks)]

    qt = np.zeros((b, 64, ncap), bf)
    ktp = np.zeros((b, 128, npairs * 128), bf)
    vext = np.zeros((b, 128, mchunks * 65), bf)
    for i in range(b):
        nq, nk = len(idx_q[i]), len(idx_k[i])
        qt[i, :, :nq] = queries[i, idx_q[i]].T.astype(bf)
        kc_t = np.zeros((64, mcap), np.float32)
        kc_t[:, :nk] = keys[i, idx_k[i]].T
        kc_t = kc_t.astype(bf)
        for p in range(npairs):
            ktp[i, 0:64, p * 128:(p + 1) * 128] = \
                kc_t[:, (2 * p) * 128:(2 * p + 1) * 128]
            if 2 * p + 1 < mchunks:
                ktp[i, 64:128, p * 128:(p + 1) * 128] = \
                    kc_t[:, (2 * p + 1) * 128:(2 * p + 2) * 128]
        ve = np.zeros((mcap, 65), np.float32)
        ve[:nk, :dv] = values[i, idx_k[i]]
        ve[:nk, dv] = 1.0
        # partition-major: [chunk, row] -> [row_in_chunk, chunk*65+col]
        vext[i] = ve.reshape(mchunks, 128, 65).transpose(1, 0, 2) \
                    .reshape(128, mchunks * 65).astype(bf)

    nc = _build(ncap, mcap)

    from concourse.bass_utils import run_bass_kernel_spmd
    in_maps = [{"qt": qt[i], "ktp": ktp[i], "vext": vext[i]} for i in range(b)]
    res = run_bass_kernel_spmd(nc, in_maps, core_ids=list(range(N_CORES)),
                               trace=trace)

    out = np.empty((b, n, dv), np.float32)
    for i in range(b):
        ot = res.results[i]["out"]  # [nblocks*65, W]
        nq, nk = len(idx_q[i]), len(idx_k[i])
        full = np.concatenate(
            [ot[jb * 65:(jb + 1) * 65, :bwidths[jb]]
             for jb in range(nblocks)], axis=1)
        num = full[:dv, :nq]
        den = full[dv, :nq]
        if nk > 0:
            out[i, :, :] = values[i, idx_k[i]].mean(axis=0)
        else:
            out[i, :, :] = 0.0
        if nq > 0:
            out[i, idx_q[i], :] = (num / den).T
    return out, res


def kernel(**inputs):
    out, _ = _run(inputs, trace=False)
    return out
